# revision 1
# baseline (speedup 1.0000x reference)
"""Causal depthwise Conv1d (K=4) + SiLU on 8 Trainium2 NeuronCores.

Problem: x (4, 8192, 2048) f32, w (2048, 1, 4) f32 ->
         y = silu(causal_depthwise_conv1d(x, w)) (4, 8192, 2048) f32.

Sharding: pure data parallel over (batch, seq-half): core c handles batch c//2,
seq rows [ (c%2)*4096, (c%2)*4096+4096 ). The K-1=3 halo is materialized on the
host (each shard ships 4099 seq positions), so cores are fully independent.

Per-core layout: host transposes the shard to (D, S) = (2048, 4099) so that DMA
reads/writes are contiguous along the free (seq) dimension. On-chip: channels on
the 128 partitions, seq on the free dim. The conv is computed on the TensorEngine
as 4 accumulating matmuls with 128x128 diagonal weight matrices (one per tap) in
float32r (full-rate fp32 PE mode), accumulated in PSUM in fp32; the ScalarEngine
applies SiLU while moving PSUM -> SBUF.
"""

import numpy as np

import concourse.bass as bass  # noqa: F401  (registers bass_rust bindings)
import concourse.mybir as mybir
import concourse.tile as tile
from concourse import bacc
from concourse.bass_utils import run_bass_kernel_spmd

B, S, D, K = 4, 8192, 2048, 4
NCORES = 8
SH = S // 2            # seq rows per core
SPAD = SH + K - 1      # shard seq width incl. halo
P = 128                # SBUF partitions
DB = D // P            # channel blocks per core
TS = 2048              # seq super-tile (1 MiB y DMA)
NT = SH // TS
CH = 512               # PSUM chunk (one bank, fp32 matmul max free dim)
NCH = TS // CH

LAST_RESULTS = None    # BassKernelResults of the most recent run (for test.py)

_cached_nc = None


def _build():
    global _cached_nc
    if _cached_nc is not None:
        return _cached_nc

    f32 = mybir.dt.float32
    f32r = mybir.dt.float32r

    nc = bacc.Bacc(
        trn_type="TRN2",
        target_bir_lowering=False,
        debug=False,
        num_devices=NCORES,
    )
    # Inputs are declared float32r (same bits as f32) so the PE can consume
    # them directly at full rate; the PE rounds on read.
    xt_d = nc.dram_tensor("xt", [D, SPAD], f32r, kind="ExternalInput").ap()
    wd_d = nc.dram_tensor("wd", [P, DB * K * P], f32r, kind="ExternalInput").ap()
    yt_d = nc.dram_tensor("yt", [D, SH], f32, kind="ExternalOutput").ap()

    with tile.TileContext(nc) as tc:
        with (
            tc.tile_pool(name="wp", bufs=1) as wpool,
            tc.tile_pool(name="xp", bufs=4) as xpool,
            tc.tile_pool(name="yp", bufs=4) as ypool,
            tc.tile_pool(name="ps", bufs=6, space="PSUM") as pspool,
        ):
            wsb = wpool.tile([P, DB * K * P], f32r)
            nc.sync.dma_start(wsb[:], wd_d)

            for j in range(DB):
                for u in range(NT):
                    xt_t = xpool.tile([P, TS + K - 1], f32r)
                    nc.sync.dma_start(
                        xt_t[:],
                        xt_d[j * P : (j + 1) * P, u * TS : u * TS + TS + K - 1],
                    )
                    y_t = ypool.tile([P, TS], f32)
                    for v in range(NCH):
                        ps = pspool.tile([P, CH], f32)
                        for k in range(K):
                            c0 = (j * K + k) * P
                            nc.tensor.matmul(
                                ps[:],
                                wsb[:, c0 : c0 + P],
                                xt_t[:, v * CH + k : v * CH + k + CH],
                                start=(k == 0),
                                stop=(k == K - 1),
                            )
                        nc.scalar.activation(
                            y_t[:, v * CH : (v + 1) * CH],
                            ps[:],
                            mybir.ActivationFunctionType.Silu,
                        )
                    nc.sync.dma_start(
                        yt_d[j * P : (j + 1) * P, u * TS : (u + 1) * TS], y_t[:]
                    )
    nc.compile()
    _cached_nc = nc
    return nc


def _prep_weights(w: np.ndarray) -> np.ndarray:
    # wd[p, j, k, m] = w[j*128+p, 0, k] if m == p else 0
    wd = np.zeros((P, DB, K, P), dtype=np.float32)
    wjk = np.transpose(w[:, 0, :].reshape(DB, P, K), (1, 0, 2))  # (P, DB, K)
    idx = np.arange(P)
    wd[idx, :, :, idx] = wjk
    return np.ascontiguousarray(wd.reshape(P, DB * K * P))


def kernel(x: np.ndarray, w: np.ndarray) -> np.ndarray:
    global LAST_RESULTS
    nc = _build()

    x = np.asarray(x, dtype=np.float32)
    wd = _prep_weights(np.asarray(w, dtype=np.float32))

    in_maps = []
    for c in range(NCORES):
        b, h = divmod(c, 2)
        s0 = h * SH
        xt = np.zeros((D, SPAD), dtype=np.float32)
        lo = s0 - (K - 1)
        if lo < 0:
            xt[:, K - 1 - s0 :] = x[b, 0 : s0 + SH, :].T
        else:
            xt[:, :] = x[b, lo : s0 + SH, :].T
        in_maps.append({"xt": xt, "wd": wd})

    res = run_bass_kernel_spmd(nc, in_maps, core_ids=list(range(NCORES)))
    LAST_RESULTS = res

    y = np.empty((B, S, D), dtype=np.float32)
    for c in range(NCORES):
        b, h = divmod(c, 2)
        s0 = h * SH
        y[b, s0 : s0 + SH, :] = res.results[c]["yt"].T
    return y


# revision 2
# speedup vs baseline: 1.5186x; 1.5186x over previous
"""Causal depthwise Conv1d (K=4) + SiLU on 8 Trainium2 NeuronCores.

Problem: x (4, 8192, 2048) f32, w (2048, 1, 4) f32 ->
         y = silu(causal_depthwise_conv1d(x, w)) (4, 8192, 2048) f32.

Sharding: pure data parallel over (batch, seq-half): core c handles batch c//2,
seq rows [ (c%2)*4096, (c%2)*4096+4096 ). The K-1=3 halo is shipped with each
shard (4099 seq positions), so cores are fully independent — no collectives.

Per-core layout: the host transposes each shard to (D, S) = (2048, 4099) so
DMAs are contiguous along the free (seq) dim. On chip: channels on the 128
partitions, seq on the free dim. The conv runs on the TensorEngine as 4
accumulating matmuls per tile with 128x128 diagonal weight matrices (one per
tap) in float32r (full-rate fp32 PE mode, ~2^-12 rounding), accumulated in
PSUM in fp32; the ScalarEngine applies SiLU on the PSUM -> SBUF move.

The diagonal weight matrices are built on-device (DVE tensor_scalar_mul of a
DMA'd 128x128 identity by per-partition weight columns) so only ~96 KB of
weight data crosses HBM instead of 4 MB. x loads use the SP HWDGE ring; y
stores use the gpsimd SWDGE path so descriptor generation for loads and
stores proceeds in parallel. A dummy Silu at kernel start preloads the ACT
table set (~2.7 us) under the pipeline fill.

TimelineSim: ~193 us/core vs a ~187 us DMA roofline (67 MB @ 360 GB/s).
"""

import numpy as np

import concourse.bass as bass  # noqa: F401  (registers bass_rust bindings)
import concourse.mybir as mybir
import concourse.tile as tile
from concourse import bacc
from concourse.bass_utils import run_bass_kernel_spmd

B, S, D, K = 4, 8192, 2048, 4
NCORES = 8
SH = S // 2            # seq rows per core
SPAD = SH + K - 1      # shard seq width incl. halo
P = 128                # SBUF partitions
DB = D // P            # channel blocks per core
TS = 512               # seq tile (= one PSUM bank of fp32)
NTILE = SH // TS

LAST_RESULTS = None    # BassKernelResults of the most recent run (for test.py)

_cached_nc = None


def _build():
    global _cached_nc
    if _cached_nc is not None:
        return _cached_nc

    f32 = mybir.dt.float32
    f32r = mybir.dt.float32r

    nc = bacc.Bacc(
        trn_type="TRN2",
        target_bir_lowering=False,
        debug=False,
        num_devices=NCORES,
    )
    # x is declared float32r (same bits as f32); the PE rounds on read.
    xt_d = nc.dram_tensor("xt", [D, SPAD], f32r, kind="ExternalInput").ap()
    eye_d = nc.dram_tensor("eye", [P, P], f32, kind="ExternalInput").ap()
    wc_d = nc.dram_tensor("wc", [P, DB * K], f32, kind="ExternalInput").ap()
    yt_d = nc.dram_tensor("yt", [D, SH], f32, kind="ExternalOutput").ap()

    with tile.TileContext(nc) as tc:
        with (
            tc.tile_pool(name="wp", bufs=1) as wpool,
            tc.tile_pool(name="xp", bufs=16) as xpool,
            tc.tile_pool(name="yp", bufs=16) as ypool,
            tc.tile_pool(name="ps", bufs=8, space="PSUM") as pspool,
        ):
            eye_t = wpool.tile([P, P], f32)
            nc.sync.dma_start(eye_t[:], eye_d)
            wc_t = wpool.tile([P, DB * K], f32)
            nc.sync.dma_start(wc_t[:], wc_d)

            # Preload the Silu ACT table set under the pipeline fill.
            scratch = wpool.tile([P, 1], f32)
            nc.vector.memset(scratch[:], 0.0)
            nc.scalar.activation(scratch[:], scratch[:],
                                 mybir.ActivationFunctionType.Silu)

            # Build the 64 diagonal 128x128 tap matrices: diag(w[j*128:+128, 0, k]).
            wsb = wpool.tile([P, DB * K * P], f32r)
            for jk in range(DB * K):
                nc.vector.tensor_scalar_mul(
                    wsb[:, jk * P:(jk + 1) * P], eye_t[:], wc_t[:, jk:jk + 1])

            for j in range(DB):
                for u in range(NTILE):
                    xt_t = xpool.tile([P, TS + K - 1], f32r)
                    nc.sync.dma_start(
                        xt_t[:],
                        xt_d[j * P:(j + 1) * P, u * TS: u * TS + TS + K - 1],
                    )
                    y_t = ypool.tile([P, TS], f32)
                    ps = pspool.tile([P, TS], f32)
                    for k in range(K):
                        c0 = (j * K + k) * P
                        nc.tensor.matmul(
                            ps[:],
                            wsb[:, c0:c0 + P],
                            xt_t[:, k: k + TS],
                            start=(k == 0),
                            stop=(k == K - 1),
                        )
                    nc.scalar.activation(
                        y_t[:], ps[:], mybir.ActivationFunctionType.Silu)
                    nc.gpsimd.dma_start(
                        yt_d[j * P:(j + 1) * P, u * TS:(u + 1) * TS], y_t[:])
    nc.compile()
    _cached_nc = nc
    return nc


def kernel(x: np.ndarray, w: np.ndarray) -> np.ndarray:
    global LAST_RESULTS
    nc = _build()

    x = np.asarray(x, dtype=np.float32)
    w = np.asarray(w, dtype=np.float32)

    eye = np.eye(P, dtype=np.float32)
    # wc[p, j*K + k] = w[j*128 + p, 0, k]
    wc = np.ascontiguousarray(
        w[:, 0, :].reshape(DB, P, K).transpose(1, 0, 2).reshape(P, DB * K))

    in_maps = []
    for c in range(NCORES):
        b, h = divmod(c, 2)
        s0 = h * SH
        xt = np.zeros((D, SPAD), dtype=np.float32)
        lo = s0 - (K - 1)
        if lo < 0:
            xt[:, K - 1 - s0:] = x[b, 0: s0 + SH, :].T
        else:
            xt[:, :] = x[b, lo: s0 + SH, :].T
        in_maps.append({"xt": xt, "eye": eye, "wc": wc})

    res = run_bass_kernel_spmd(nc, in_maps, core_ids=list(range(NCORES)))
    LAST_RESULTS = res

    y = np.empty((B, S, D), dtype=np.float32)
    for c in range(NCORES):
        b, h = divmod(c, 2)
        s0 = h * SH
        y[b, s0: s0 + SH, :] = res.results[c]["yt"].T
    return y


# revision 8
# speedup vs baseline: 1.5258x; 1.0047x over previous
"""Causal depthwise Conv1d (K=4) + SiLU on 8 Trainium2 NeuronCores.

Problem: x (4, 8192, 2048) f32, w (2048, 1, 4) f32 ->
         y = silu(causal_depthwise_conv1d(x, w)) (4, 8192, 2048) f32.

Sharding: pure data parallel over (batch, seq-half): core c handles batch c//2,
seq rows [ (c%2)*4096, (c%2)*4096+4096 ). The K-1=3 halo is shipped with each
shard (4099 seq positions), so cores are fully independent — no collectives.

Per-core layout: the host transposes each shard to (D, S) = (2048, 4099) so
DMAs are contiguous along the free (seq) dim. On chip: channels on the 128
partitions, seq on the free dim. The conv runs on the TensorEngine as 4
accumulating matmuls per tile with 128x128 diagonal weight matrices (one per
tap) in float32r (full-rate fp32 PE mode, ~2^-12 rounding), accumulated in
PSUM in fp32; the ScalarEngine applies SiLU on the PSUM -> SBUF move.

The diagonal weight matrices are built on-device (DVE tensor_scalar_mul of a
DMA'd 128x128 identity by per-partition weight columns) so only ~96 KB of
weight data crosses HBM instead of 4 MB. x loads use the SP HWDGE ring; y
stores alternate between the gpsimd SWDGE path and the ACT HWDGE ring so
descriptor generation for loads and stores proceeds in parallel. A dummy Silu
at kernel start preloads the ACT table set (~2.7 us) under the pipeline fill.

TimelineSim: ~192 us/core vs a ~187 us DMA roofline (67 MB @ 360 GB/s).

Execution uses a locally-cached jax.jit(shard_map) built once per process
(bass2jax.run_bass_via_pjrt rebuilds and retraces it per call).
"""

import time

import numpy as np

import concourse.bass as bass  # noqa: F401  (registers bass_rust bindings)
import concourse.mybir as mybir
import concourse.tile as tile
from concourse import bacc

B, S, D, K = 4, 8192, 2048, 4
NCORES = 8
SH = S // 2            # seq rows per core
SPAD = SH + K - 1      # shard seq width incl. halo
P = 128                # SBUF partitions
DB = D // P            # channel blocks per core
TS = 512               # seq tile (= one PSUM bank of fp32)
NTILE = SH // TS

VERBOSE = False        # set by test.py for phase timings

_cached = None         # cached jitted runner
_cached_nc = None      # cached compiled Bass program


def _build_nc():
    global _cached_nc
    if _cached_nc is not None:
        return _cached_nc
    f32 = mybir.dt.float32
    f32r = mybir.dt.float32r

    nc = bacc.Bacc(
        trn_type="TRN2",
        target_bir_lowering=False,
        debug=False,
        num_devices=NCORES,
    )
    # x is declared float32r (same bits as f32); the PE rounds on read.
    xt_d = nc.dram_tensor("xt", [D, SPAD], f32r, kind="ExternalInput").ap()
    eye_d = nc.dram_tensor("eye", [P, P], f32, kind="ExternalInput").ap()
    wc_d = nc.dram_tensor("wc", [P, DB * K], f32, kind="ExternalInput").ap()
    yt_d = nc.dram_tensor("yt", [D, SH], f32, kind="ExternalOutput").ap()

    with tile.TileContext(nc) as tc:
        with (
            tc.tile_pool(name="wp", bufs=1) as wpool,
            tc.tile_pool(name="xp", bufs=16) as xpool,
            tc.tile_pool(name="yp", bufs=16) as ypool,
            tc.tile_pool(name="ps", bufs=8, space="PSUM") as pspool,
        ):
            eye_t = wpool.tile([P, P], f32)
            nc.sync.dma_start(eye_t[:], eye_d)
            wc_t = wpool.tile([P, DB * K], f32)
            nc.sync.dma_start(wc_t[:], wc_d)

            # Preload the Silu ACT table set under the pipeline fill.
            scratch = wpool.tile([P, 1], f32)
            nc.vector.memset(scratch[:], 0.0)
            nc.scalar.activation(scratch[:], scratch[:],
                                 mybir.ActivationFunctionType.Silu)

            # Build the 64 diagonal 128x128 tap matrices: diag(w[j*128:+128, 0, k]).
            wsb = wpool.tile([P, DB * K * P], f32r)
            for jk in range(DB * K):
                nc.vector.tensor_scalar_mul(
                    wsb[:, jk * P:(jk + 1) * P], eye_t[:], wc_t[:, jk:jk + 1])

            n = 0
            for j in range(DB):
                for u in range(NTILE):
                    xt_t = xpool.tile([P, TS + K - 1], f32r)
                    nc.sync.dma_start(
                        xt_t[:],
                        xt_d[j * P:(j + 1) * P, u * TS: u * TS + TS + K - 1],
                    )
                    y_t = ypool.tile([P, TS], f32)
                    ps = pspool.tile([P, TS], f32)
                    for k in range(K):
                        c0 = (j * K + k) * P
                        nc.tensor.matmul(
                            ps[:],
                            wsb[:, c0:c0 + P],
                            xt_t[:, k: k + TS],
                            start=(k == 0),
                            stop=(k == K - 1),
                        )
                    nc.scalar.activation(
                        y_t[:], ps[:], mybir.ActivationFunctionType.Silu)
                    y_eng = nc.gpsimd if n % 2 == 0 else nc.scalar
                    y_eng.dma_start(
                        yt_d[j * P:(j + 1) * P, u * TS:(u + 1) * TS], y_t[:])
                    n += 1
    nc.compile()
    _cached_nc = nc
    return nc


def _get_runner():
    """Build (once) a cached jax.jit(shard_map) executing the Bass program on
    8 cores. Mirrors bass2jax.run_bass_via_pjrt's multi-core path, but the
    jitted callable survives across kernel() calls (the library rebuilds and
    retraces it per invocation)."""
    global _cached
    if _cached is not None:
        return _cached

    import jax
    from jax.sharding import Mesh, PartitionSpec
    from jax.experimental.shard_map import shard_map
    from concourse import bass2jax

    bass2jax.install_neuronx_cc_hook()

    nc = _build_nc()

    in_names = ["xt", "eye", "wc"]
    out_names = ["yt"]
    out_avals = (jax.core.ShapedArray((D, SH), np.float32),)
    all_names = in_names + out_names + ["partition_id"]
    n_params = len(in_names)

    def _body(*args):
        operands = list(args)
        operands.append(bass2jax.partition_id_tensor())
        outs = bass2jax._bass_exec_p.bind(
            *operands,
            out_avals=out_avals,
            in_names=tuple(all_names),
            out_names=tuple(out_names),
            lowering_input_output_aliases=(),
            sim_require_finite=True,
            sim_require_nnan=True,
            nc=nc,
        )
        return tuple(outs)

    devices = jax.devices()[:NCORES]
    mesh = Mesh(np.asarray(devices), ("core",))
    n_args = n_params + len(out_names)
    sharded = jax.jit(
        shard_map(
            _body,
            mesh=mesh,
            in_specs=(PartitionSpec("core"),) * n_args,
            out_specs=(PartitionSpec("core"),) * len(out_names),
            check_rep=False,
        ),
        donate_argnums=(n_params,),
        keep_unused=True,
    )
    _cached = sharded
    return sharded


def kernel(x: np.ndarray, w: np.ndarray) -> np.ndarray:
    t0 = time.time()
    sharded = _get_runner()
    t_build = time.time() - t0

    x = np.asarray(x, dtype=np.float32)
    w = np.asarray(w, dtype=np.float32)

    t0 = time.time()
    eye = np.broadcast_to(np.eye(P, dtype=np.float32), (NCORES, P, P)).reshape(
        NCORES * P, P)
    # wc[p, j*K + k] = w[j*128 + p, 0, k]
    wc1 = np.ascontiguousarray(
        w[:, 0, :].reshape(DB, P, K).transpose(1, 0, 2).reshape(P, DB * K))
    wc = np.broadcast_to(wc1, (NCORES, P, DB * K)).reshape(NCORES * P, DB * K)

    # Concatenated per-core transposed shards: (8*2048, 4099)
    xt = np.zeros((NCORES * D, SPAD), dtype=np.float32)
    for c in range(NCORES):
        b, h = divmod(c, 2)
        s0 = h * SH
        lo = s0 - (K - 1)
        dst = xt[c * D:(c + 1) * D]
        if lo < 0:
            dst[:, K - 1 - s0:] = x[b, 0: s0 + SH, :].T
        else:
            dst[:, :] = x[b, lo: s0 + SH, :].T
    zeros = np.zeros((NCORES * D, SH), dtype=np.float32)
    t_prep = time.time() - t0

    t0 = time.time()
    (out,) = sharded(xt, eye, wc, zeros)
    t_run = time.time() - t0

    t0 = time.time()
    out_np = np.asarray(out).reshape(NCORES, D, SH)
    y = np.empty((B, S, D), dtype=np.float32)
    for c in range(NCORES):
        b, h = divmod(c, 2)
        s0 = h * SH
        y[b, s0: s0 + SH, :] = out_np[c].T
    t_post = time.time() - t0

    if VERBOSE:
        print(f"[kernel] build {t_build:.2f}s prep {t_prep:.2f}s "
              f"run {t_run:.2f}s post {t_post:.2f}s", flush=True)
    return y
